# revision 4
# baseline (speedup 1.0000x reference)
"""Trainium2 Bass kernel for the BQNN boson-sampling simulation.

Strategy: pure data parallel over 8 NeuronCores (batch 32768 -> 8 x 4096).
Per core, batch maps to [128 partitions x 32 free slots]. Only the first 3
columns of U_final matter (IN_IDX = [0,1,2]), so the 10 MZI rotations are
applied directly to S3 = start[:, 0:3] (6x3 complex, per batch element),
then the 20 permanents are computed via expansion along column 0:
    perm(i,j,k) = x_i*P[jk] + x_j*P[ik] + x_k*P[ij],  P[ab] = y_a*z_b + y_b*z_a
All parameter-only math (static Clements unitary, constant-rotation
coefficients, affine constants) is folded on the host into a 96-float
runtime input, so the Bass program compiles once.
"""

import math
import numpy as np

import concourse.bass as bass
import concourse.mybir as mybir
from concourse.tile import TileContext
from concourse.bass_utils import run_bass_kernel_spmd

F32 = mybir.dt.float32
I32 = mybir.dt.int32
ALU = mybir.AluOpType
ACTF = mybir.ActivationFunctionType

N_CORES = 8
BATCH = 32768
SHARD = BATCH // N_CORES          # 4096
P = 128                           # partitions
T = SHARD // P                    # 32 free slots per partition
TWO_PI = 2.0 * math.pi


def _sin_poly_coeffs():
    """L2 fit of sin(s)/s as poly in z=s^2 on [-pi,pi], weighted by |s|."""
    s = np.linspace(-math.pi, math.pi, 40000)  # even: avoids s=0
    z = s * s
    A = np.stack([z ** k for k in range(6)], axis=1)
    w = np.abs(s)
    Aw = A * w[:, None]
    bw = (np.sin(s) / s) * w
    c, *_ = np.linalg.lstsq(Aw, bw, rcond=None)
    return [float(v) for v in c]


SINC = _sin_poly_coeffs()

# Clements mesh (static interferometer) mode pairs, 15 MZIs
def _clements_modes(n=6):
    pairs = []
    for layer in range(n):
        start = 0 if layer % 2 == 0 else 1
        for m in range(start, n - 1, 2):
            pairs.append((m, m + 1))
    return pairs

CLEMENTS_MODES = _clements_modes(6)
ANSATZ_MODES = [(0, 1), (2, 3), (4, 5), (1, 2), (3, 4)] * 2  # 10 MZIs
DATA_ROTS = [0, 1, 2, 5, 6, 7]
CONST_ROTS = [3, 4, 8, 9]

PAIRS = [(j, k) for j in range(6) for k in range(j + 1, 6)]          # 15, lex
PAIR_IDX = {p: i for i, p in enumerate(PAIRS)}
TRIPLES = [(i, j, k) for i in range(6) for j in range(i + 1, 6)
           for k in range(j + 1, 6)]                                  # 20, lex

# ---------------------------------------------------------------- host math

def _calc_start_cols(params, output_phase):
    """Static 6x6 Clements unitary with output phases; returns cols 0..2."""
    phi = np.asarray(params[0:15], dtype=np.float32)
    theta = np.asarray(params[15:30], dtype=np.float32)
    U = np.eye(6, dtype=np.complex64)
    for k, (m, n) in enumerate(CLEMENTS_MODES):
        ct = np.complex64(np.cos(theta[k], dtype=np.float32))
        st = np.complex64(np.sin(theta[k], dtype=np.float32))
        ep = np.exp(1j * np.complex64(phi[k]))
        Tm = np.eye(6, dtype=np.complex64)
        Tm[m, m] = ep * ct
        Tm[m, n] = -st
        Tm[n, m] = ep * st
        Tm[n, n] = ct
        U = Tm @ U
    D = np.diag(np.exp(1j * np.asarray(output_phase, dtype=np.float32)
                       .astype(np.complex64)))
    U = D @ U
    return U[:, 0:3]  # [6,3] complex64

# D-vector layout (ND floats, broadcast to every partition on chip)
ND = 96
IDX_S3R = 0          # 18: s3 real, (row*3+col)
IDX_S3I = 18         # 18: s3 imag
IDX_CR = 36          # 4 const rots x 8: [ar, ai, nai, gr, gi, ngi, ct, nst]
IDX_K = 68           # 12: input_k
IDX_B = 80           # 12: input_b + 64*pi  (positivity shift for mod trick)


def _build_dvec(params, output_phase, param_phi, param_theta, input_k, input_b):
    d = np.zeros(ND, dtype=np.float32)
    s3 = _calc_start_cols(params, output_phase)
    d[IDX_S3R:IDX_S3R + 18] = np.real(s3).astype(np.float32).reshape(-1)
    d[IDX_S3I:IDX_S3I + 18] = np.imag(s3).astype(np.float32).reshape(-1)
    # const rotations: ansatz indices 3,4,8,9 use param slots 0,1,2,3
    for q, k in enumerate(CONST_ROTS):
        slot = {3: 0, 4: 1, 8: 2, 9: 3}[k]
        ph = np.float32(param_phi[slot])
        th = np.float32(param_theta[slot])
        ct = np.float32(np.cos(th)); st = np.float32(np.sin(th))
        cp = np.float32(np.cos(ph)); sp = np.float32(np.sin(ph))
        base = IDX_CR + 8 * q
        d[base + 0] = cp * ct        # ar
        d[base + 1] = sp * ct        # ai
        d[base + 2] = -(sp * ct)     # nai
        d[base + 3] = cp * st        # gr
        d[base + 4] = sp * st        # gi
        d[base + 5] = -(sp * st)     # ngi
        d[base + 6] = ct             # ct
        d[base + 7] = -st            # nst
    d[IDX_K:IDX_K + 12] = np.asarray(input_k, dtype=np.float32)
    d[IDX_B:IDX_B + 12] = (np.asarray(input_b, dtype=np.float32)
                           + np.float32(64.0 * math.pi))
    return d

# ---------------------------------------------------------------- AP helpers

def _ap3(tile_ap, base, nblk, blk_step, tile=None):
    """[128, nblk, 32] AP into a tile starting at free column `base`."""
    a = tile_ap if tile is None else tile[:, 0:1]
    return bass.AP(a.tensor, a.offset + base, [a.ap[0], [blk_step, nblk], [1, T]])


def _bc(tile, col, nblk):
    """broadcast block `col` of a [128, n*32] tile nblk times (step-0)."""
    a = tile[:, col * T:(col + 1) * T]
    return bass.AP(a.tensor, a.offset, [a.ap[0], [0, nblk], [1, T]])


def _bcs(tile, col, nblk):
    """broadcast strided column j of a row-major (t,12) tile, nblk times."""
    a = tile[:, 0:1]
    return bass.AP(a.tensor, a.offset + col, [a.ap[0], [0, nblk], [12, T]])


def _blk(tile, start_blk, nblk):
    """contiguous blocks [start, start+nblk) of a tile as [128, nblk, 32]."""
    a = tile[:, 0:1]
    return bass.AP(a.tensor, a.offset + start_blk * T,
                   [a.ap[0], [T, nblk], [1, T]])

# ---------------------------------------------------------------- bass build

def build_kernel(reps=1, split_waits=True):
    nc = bass.Bass()
    xd_ext = nc.declare_dram_parameter("xd", [P, 12 * T + ND + 24 * T], F32,
                                       isOutput=False)
    vi_ext = nc.declare_dram_parameter("vinit", [P, 36 * T], F32,
                                       isOutput=False)
    out_ext = nc.declare_dram_parameter("out", [SHARD, 20], F32, isOutput=True)

    with TileContext(nc) as tc:
        with tc.tile_pool(name="main", bufs=1) as pool, \
             tc.tile_pool(name="scr", bufs=2) as scr:
            XD = pool.tile([P, 12 * T + ND + 24 * T], F32, name="XD", tag="XD")
            VV0 = None
            for _rep in range(reps):
                VV = pool.tile([P, 36 * T], F32, name="VV", tag="VV")
                XS = pool.tile([P, 12 * T], F32, name="XS", tag="XS")

                # --- DMA in (V init data; XD loaded once before the loop) ---
                if _rep == 0:
                    nc.sync.dma_start(out=XD[:, :], in_=xd_ext[:, :])
                nc.sync.dma_start(out=VV[:, :], in_=vi_ext[:, :])
                DOF = 12 * T  # offset of scalars within XD
                KOF = DOF + ND
                BOF = KOF + 12 * T

                def dsc(i):
                    return XD[:, DOF + i:DOF + i + 1]

                # --- affine (row-major (t,j)): XS = x*K_full + B_full ---
                def dscb(i):
                    a = XD[:, DOF + i:DOF + i + 1]
                    return bass.AP(a.tensor, a.offset, [a.ap[0], [0, T]])

                nc.vector.tensor_tensor(
                    XS[:, :], XD[:, 0:12 * T], XD[:, KOF:KOF + 12 * T],
                    ALU.mult)
                nc.vector.tensor_tensor(
                    XS[:, :], XS[:, :], XD[:, BOF:BOF + 12 * T], ALU.add)

                # --- range-reduce to [-pi, pi]; sin via DVE polynomial ---
                W2 = 24 * T
                XS2 = scr.tile([P, W2], F32, name="xs2", tag="xs2")
                nc.gpsimd.tensor_copy(XS2[:, 0:12 * T], XS[:, :])
                nc.vector.tensor_scalar_add(XS2[:, 12 * T:W2], XS[:, :],
                                            math.pi / 2.0)
                SR = scr.tile([P, W2], F32, name="sr", tag="sr")
                y = scr.tile([P, W2], F32, name="rr_y", tag="rr_y")
                nc.vector.tensor_scalar_mul(y[:, :], XS2[:, :], 1.0 / TWO_PI)
                yi = scr.tile([P, W2], I32, name="rr_i", tag="rr_i")
                nc.gpsimd.tensor_copy(yi[:, :], y[:, :])
                yf = scr.tile([P, W2], F32, name="rr_f", tag="rr_f")
                nc.gpsimd.tensor_copy(yf[:, :], yi[:, :])
                nc.vector.scalar_tensor_tensor(
                    SR[:, :], yf[:, :], -TWO_PI, XS2[:, :], ALU.mult, ALU.add)
                g = scr.tile([P, W2], F32, name="rr_g", tag="rr_g")
                nc.vector.tensor_scalar(
                    g[:, :], SR[:, :], math.pi, None, ALU.is_gt)
                nc.vector.scalar_tensor_tensor(
                    SR[:, :], g[:, :], -TWO_PI, SR[:, :], ALU.mult, ALU.add)
                # sin via the scalar engine (range already reduced)
                SINCOS = pool.tile([P, W2], F32, name="SINCOS", tag="SINCOS")
                nc.scalar.activation(SINCOS[:, :], SR[:, :],
                                     ACTF.Sin)

                # row pointers: (tile, base_col) per row/component
                rowr = [(VV, r * 3 * T) for r in range(6)]
                rowi = [(VV, 18 * T + r * 3 * T) for r in range(6)]

                def rr(m):
                    t, b = rowr[m]
                    return _ap3(t[:, 0:1], b, 3, T)

                def ri(m):
                    t, b = rowi[m]
                    return _ap3(t[:, 0:1], b, 3, T)

                def s3d(tile):
                    return _ap3(tile[:, 0:1], 0, 3, T)

                # --- fused data layer: rots (k0..k0+2) on disjoint row pairs;
                # every operand is a 4D AP shaped (rot 3, col 3, T) so the
                # engine iterates identically on all of them.
                def data_layer(k0):
                    layer = 0 if k0 < 3 else 1
                    pcol0 = layer * 6
                    tcol0 = layer * 6 + 3

                    def coef(col0):
                        a = SINCOS[:, 0:1]
                        return bass.AP(a.tensor, a.offset + col0,
                                       [a.ap[0], [1, 3], [0, 3], [12, T]])

                    cp = coef(12 * T + pcol0)
                    sp = coef(pcol0)
                    ct = coef(12 * T + tcol0)
                    st = coef(tcol0)

                    def vap(comp, row0):
                        a = VV[:, 0:1]
                        return bass.AP(a.tensor,
                                       a.offset + comp * 18 * T + row0 * 3 * T,
                                       [a.ap[0], [6 * T, 3], [T, 3], [1, T]])

                    vmr, vmi = vap(0, 0), vap(1, 0)
                    vnr, vni = vap(0, 1), vap(1, 1)

                    def tmp9(tag):
                        t9 = scr.tile([P, 9 * T], F32, name=tag, tag=tag)
                        a = t9[:, 0:1]
                        return bass.AP(a.tensor, a.offset,
                                       [a.ap[0], [3 * T, 3], [T, 3], [1, T]])

                    t1 = tmp9("dl_t1"); t2 = tmp9("dl_t2")
                    wr = tmp9("dl_wr"); wi = tmp9("dl_wi")
                    nc.vector.tensor_tensor(t1, cp, vmr, ALU.mult)
                    nc.vector.tensor_tensor(t2, sp, vmi, ALU.mult)
                    nc.vector.tensor_tensor(wr, t1, t2, ALU.subtract)
                    nc.vector.tensor_tensor(t1, cp, vmi, ALU.mult)
                    nc.vector.tensor_tensor(t2, sp, vmr, ALU.mult)
                    nc.vector.tensor_tensor(wi, t1, t2, ALU.add)
                    u1 = tmp9("dl_u1"); u2 = tmp9("dl_u2")
                    nc.vector.tensor_tensor(u1, ct, wr, ALU.mult)
                    nc.vector.tensor_tensor(u2, st, vnr, ALU.mult)
                    nc.vector.tensor_tensor(vmr, u1, u2, ALU.subtract)
                    nc.vector.tensor_tensor(u1, ct, wi, ALU.mult)
                    nc.vector.tensor_tensor(u2, st, vni, ALU.mult)
                    nc.vector.tensor_tensor(vmi, u1, u2, ALU.subtract)
                    nc.vector.tensor_tensor(u1, st, wr, ALU.mult)
                    nc.vector.tensor_tensor(u2, ct, vnr, ALU.mult)
                    nc.vector.tensor_tensor(vnr, u1, u2, ALU.add)
                    nc.vector.tensor_tensor(u1, st, wi, ALU.mult)
                    nc.vector.tensor_tensor(u2, ct, vni, ALU.mult)
                    nc.vector.tensor_tensor(vni, u1, u2, ALU.add)

                # --- data rotation: Vm' = ct*(e^{ip}Vm) - st*Vn ; Vn' = st*W + ct*Vn
                def data_rot(k):
                    m, n = ANSATZ_MODES[k]
                    layer = 0 if k < 3 else 1
                    idx = k if k < 3 else k - 5
                    pcol = layer * 6 + idx        # phi col in xs
                    tcol = layer * 6 + 3 + idx    # theta col in xs
                    cp = _bcs(SINCOS, 12 * T + pcol, 3)
                    sp = _bcs(SINCOS, pcol, 3)
                    ct = _bcs(SINCOS, 12 * T + tcol, 3)
                    st = _bcs(SINCOS, tcol, 3)

                    def tmp(tag):
                        t = scr.tile([P, 3 * T], F32, tag=tag)
                        return t, s3d(t)

                    t1t, t1 = tmp("dr_t1"); t2t, t2 = tmp("dr_t2")
                    wrt, wr = tmp("dr_wr"); wit, wi = tmp("dr_wi")
                    nc.vector.tensor_tensor(t1, cp, rr(m), ALU.mult)
                    nc.vector.tensor_tensor(t2, sp, ri(m), ALU.mult)
                    nc.vector.tensor_tensor(wr, t1, t2, ALU.subtract)
                    nc.vector.tensor_tensor(t1, cp, ri(m), ALU.mult)
                    nc.vector.tensor_tensor(t2, sp, rr(m), ALU.mult)
                    nc.vector.tensor_tensor(wi, t1, t2, ALU.add)
                    u1t, u1 = tmp("dr_u1"); u2t, u2 = tmp("dr_u2")
                    # new Vm = ct*W - st*Vn  (write Vm in place; Vn' reads W, Vn)
                    nc.vector.tensor_tensor(u1, ct, wr, ALU.mult)
                    nc.vector.tensor_tensor(u2, st, rr(n), ALU.mult)
                    nc.vector.tensor_tensor(rr(m), u1, u2, ALU.subtract)
                    nc.vector.tensor_tensor(u1, ct, wi, ALU.mult)
                    nc.vector.tensor_tensor(u2, st, ri(n), ALU.mult)
                    nc.vector.tensor_tensor(ri(m), u1, u2, ALU.subtract)
                    # new Vn = st*W + ct*Vn
                    nc.vector.tensor_tensor(u1, st, wr, ALU.mult)
                    nc.vector.tensor_tensor(u2, ct, rr(n), ALU.mult)
                    nc.vector.tensor_tensor(rr(n), u1, u2, ALU.add)
                    nc.vector.tensor_tensor(u1, st, wi, ALU.mult)
                    nc.vector.tensor_tensor(u2, ct, ri(n), ALU.mult)
                    nc.vector.tensor_tensor(ri(n), u1, u2, ALU.add)

                # --- const rotation (coeffs are runtime [P,1] scalars from D) ---
                def const_rot(k, m_dst=None):
                    q = CONST_ROTS.index(k)
                    base = IDX_CR + 8 * q
                    ar, ai, nai = dsc(base), dsc(base + 1), dsc(base + 2)
                    gr, gi, ngi = dsc(base + 3), dsc(base + 4), dsc(base + 5)
                    ct, nst = dsc(base + 6), dsc(base + 7)
                    m, n = ANSATZ_MODES[k]
                    omr_ap, omi_ap, new_ptr_r, new_ptr_i = m_dst

                    def tmp(tag):
                        t = scr.tile([P, 3 * T], F32, tag=tag)
                        return s3d(t)

                    # m-row outputs (write to destination, not in place)
                    t = tmp("cr_t"); u = tmp("cr_u")
                    nc.vector.tensor_scalar(t, rr(m), ar, None, ALU.mult)
                    nc.vector.scalar_tensor_tensor(u, ri(m), nai, t, ALU.mult, ALU.add)
                    nc.vector.scalar_tensor_tensor(omr_ap, rr(n), nst, u, ALU.mult, ALU.add)
                    t2 = tmp("cr_t2"); u2 = tmp("cr_u2")
                    nc.vector.tensor_scalar(t2, ri(m), ar, None, ALU.mult)
                    nc.vector.scalar_tensor_tensor(u2, rr(m), ai, t2, ALU.mult, ALU.add)
                    nc.vector.scalar_tensor_tensor(omi_ap, ri(n), nst, u2, ALU.mult, ALU.add)
                    # n-row in place (after m-row reads of Vn are emitted)
                    t3 = tmp("cr_t3"); u3 = tmp("cr_u3")
                    nc.vector.tensor_scalar(t3, rr(m), gr, None, ALU.mult)
                    nc.vector.scalar_tensor_tensor(u3, ri(m), ngi, t3, ALU.mult, ALU.add)
                    nc.vector.scalar_tensor_tensor(rr(n), rr(n), ct, u3, ALU.mult, ALU.add)
                    t4 = tmp("cr_t4"); u4 = tmp("cr_u4")
                    nc.vector.tensor_scalar(t4, ri(m), gr, None, ALU.mult)
                    nc.vector.scalar_tensor_tensor(u4, rr(m), gi, t4, ALU.mult, ALU.add)
                    nc.vector.scalar_tensor_tensor(ri(n), ri(n), ct, u4, ALU.mult, ALU.add)
                    rowr[m] = new_ptr_r
                    rowi[m] = new_ptr_i

                # C1 m-rows go to scratch (pointer redirect); C2 back into V slots
                c1r = {}
                for k in (3, 4):
                    m, _ = ANSATZ_MODES[k]
                    tr = pool.tile([P, 3 * T], F32, name=f"c1r{m}", tag=f"c1r{m}")
                    ti = pool.tile([P, 3 * T], F32, name=f"c1i{m}", tag=f"c1i{m}")
                    c1r[k] = (s3d(tr), s3d(ti), (tr, 0), (ti, 0))

                data_layer(0)
                for k in (3, 4):
                    const_rot(k, m_dst=c1r[k])
                for k in (5, 6, 7):
                    data_rot(k)
                for k in (8, 9):
                    m, _ = ANSATZ_MODES[k]
                    const_rot(k, m_dst=(_ap3(VV[:, 0:1], m * 3 * T, 3, T),
                                        _ap3(VV[:, 0:1], 18 * T + m * 3 * T,
                                             3, T),
                                        (VV, m * 3 * T),
                                        (VV, 18 * T + m * 3 * T)))

                # V is now fully materialized in Vr/Vi (rows contiguous).
                # col views: x = col0, y = col1, z = col2 of each row
                def run_src(comp, row, col, nblk, stride_rows):
                    b = comp * 18 * T + (row * 3 + col) * T
                    if stride_rows:
                        return _ap3(VV[:, 0:1], b, nblk, 3 * T)
                    a = VV[:, 0:1]
                    return bass.AP(a.tensor, a.offset + b,
                                   [a.ap[0], [0, nblk], [1, T]])

                # --- P stage: P[jk] = y_j z_k + y_k z_j  (15 pairs) ---
                PW = 15 * T
                Y1 = [pool.tile([P, PW], F32, name=f"Y1{c}", tag=f"Y1{c}") for c in range(2)]
                Z1 = [pool.tile([P, PW], F32, name=f"Z1{c}", tag=f"Z1{c}") for c in range(2)]
                Y2 = [pool.tile([P, PW], F32, name=f"Y2{c}", tag=f"Y2{c}") for c in range(2)]
                Z2 = [pool.tile([P, PW], F32, name=f"Z2{c}", tag=f"Z2{c}") for c in range(2)]

                cp_engines = [nc.gpsimd, nc.scalar]
                cp_n = [0]

                def emit_copy(dst_ap, src_ap):
                    cp_n[0] += 1
                    if cp_n[0] % 3 == 0:
                        nc.gpsimd.tensor_copy(dst_ap, src_ap)
                    else:
                        nc.vector.tensor_copy(dst_ap, src_ap)

                s = 0
                for j in range(5):
                    L = 5 - j
                    for c in range(2):
                        emit_copy(_blk(Y1[c], s, L), run_src(c, j, 1, L, False))
                        emit_copy(_blk(Z1[c], s, L), run_src(c, j + 1, 2, L, True))
                        emit_copy(_blk(Y2[c], s, L), run_src(c, j + 1, 1, L, True))
                        emit_copy(_blk(Z2[c], s, L), run_src(c, j, 2, L, False))
                    s += L

                Pr = pool.tile([P, PW], F32, name="Pr", tag="Pr")
                Pi = pool.tile([P, PW], F32, name="Pi", tag="Pi")
                pa = scr.tile([P, PW], F32, name="p_a", tag="p_a")
                pb = scr.tile([P, PW], F32, name="p_b", tag="p_b")
                # products split Pool/DVE; accumulation chains on DVE
                pc2 = scr.tile([P, PW], F32, name="p_c2", tag="p_c2")
                pd2 = scr.tile([P, PW], F32, name="p_d2", tag="p_d2")
                # Pr = Y1r*Z1r - Y1i*Z1i + Y2r*Z2r - Y2i*Z2i
                nc.gpsimd.tensor_tensor(pa[:, :], Y1[0][:, :], Z1[0][:, :], ALU.mult)
                nc.gpsimd.tensor_tensor(pb[:, :], Y1[1][:, :], Z1[1][:, :], ALU.mult)
                nc.vector.tensor_tensor(pc2[:, :], Y2[0][:, :], Z2[0][:, :], ALU.mult)
                nc.vector.tensor_tensor(pd2[:, :], Y2[1][:, :], Z2[1][:, :], ALU.mult)
                nc.vector.tensor_tensor(pa[:, :], pa[:, :], pb[:, :], ALU.subtract)
                nc.vector.tensor_tensor(pa[:, :], pa[:, :], pc2[:, :], ALU.add)
                nc.vector.tensor_tensor(Pr[:, :], pa[:, :], pd2[:, :], ALU.subtract)
                # Pi = Y1r*Z1i + Y1i*Z1r + Y2r*Z2i + Y2i*Z2r
                nc.gpsimd.tensor_tensor(pa[:, :], Y1[0][:, :], Z1[1][:, :], ALU.mult)
                nc.gpsimd.tensor_tensor(pb[:, :], Y1[1][:, :], Z1[0][:, :], ALU.mult)
                nc.vector.tensor_tensor(pc2[:, :], Y2[0][:, :], Z2[1][:, :], ALU.mult)
                nc.vector.tensor_tensor(pd2[:, :], Y2[1][:, :], Z2[0][:, :], ALU.mult)
                nc.vector.tensor_tensor(pa[:, :], pa[:, :], pb[:, :], ALU.add)
                nc.vector.tensor_tensor(pa[:, :], pa[:, :], pc2[:, :], ALU.add)
                nc.vector.tensor_tensor(Pi[:, :], pa[:, :], pd2[:, :], ALU.add)

                # --- T stage gathers ---
                TW = 20 * T
                X1 = [pool.tile([P, TW], F32, name=f"X1{c}", tag=f"X1{c}") for c in range(2)]
                X2 = [pool.tile([P, TW], F32, name=f"X2{c}", tag=f"X2{c}") for c in range(2)]
                X3 = [pool.tile([P, TW], F32, name=f"X3{c}", tag=f"X3{c}") for c in range(2)]
                PA = [pool.tile([P, TW], F32, name=f"PA{c}", tag=f"PA{c}") for c in range(2)]
                PB = [pool.tile([P, TW], F32, name=f"PB{c}", tag=f"PB{c}") for c in range(2)]
                PC = [pool.tile([P, TW], F32, name=f"PC{c}", tag=f"PC{c}") for c in range(2)]

                def p_src(idx, nblk, contiguous, pt):
                    if contiguous:
                        return _blk(pt, idx, nblk)
                    return _bc(pt, idx, nblk)

                # X1 / PA: runs grouped by i
                s = 0
                for i in range(4):
                    L = (5 - i) * (4 - i) // 2
                    pstart = PAIR_IDX[(i + 1, i + 2)]
                    for c in range(2):
                        emit_copy(_blk(X1[c], s, L), run_src(c, i, 0, L, False))
                        pt = Pr if c == 0 else Pi
                        emit_copy(_blk(PA[c], s, L), p_src(pstart, L, True, pt))
                    s += L
                # X2/X3/PB/PC: runs grouped by (i,j)
                s = 0
                for i in range(4):
                    for j in range(i + 1, 5):
                        L = 5 - j
                        for c in range(2):
                            pt = Pr if c == 0 else Pi
                            emit_copy(_blk(X2[c], s, L), run_src(c, j, 0, L, False))
                            emit_copy(_blk(X3[c], s, L), run_src(c, j + 1, 0, L, True))
                            emit_copy(_blk(PB[c], s, L),
                                      p_src(PAIR_IDX[(i, j + 1)], L, True, pt))
                            emit_copy(_blk(PC[c], s, L),
                                      p_src(PAIR_IDX[(i, j)], L, False, pt))
                        s += L

                # --- T compute: A = X1*PA + X2*PB + X3*PC (complex) ---
                # 12 independent products go to Pool; DVE only runs the
                # add/subtract accumulation chains.
                Ar = pool.tile([P, TW], F32, name="Ar", tag="Ar")
                Ai = pool.tile([P, TW], F32, name="Ai", tag="Ai")
                tp = [scr.tile([P, TW], F32, name=f"t_p{i}", tag=f"t_p{i}")
                      for i in range(6)]
                ta = scr.tile([P, TW], F32, name="t_a", tag="t_a")
                tb = scr.tile([P, TW], F32, name="t_b", tag="t_b")
                # real products
                nc.gpsimd.tensor_tensor(tp[0][:, :], X1[0][:, :], PA[0][:, :], ALU.mult)
                nc.gpsimd.tensor_tensor(tp[1][:, :], X1[1][:, :], PA[1][:, :], ALU.mult)
                nc.gpsimd.tensor_tensor(tp[2][:, :], X2[0][:, :], PB[0][:, :], ALU.mult)
                nc.vector.tensor_tensor(tp[3][:, :], X2[1][:, :], PB[1][:, :], ALU.mult)
                nc.vector.tensor_tensor(tp[4][:, :], X3[0][:, :], PC[0][:, :], ALU.mult)
                nc.vector.tensor_tensor(tp[5][:, :], X3[1][:, :], PC[1][:, :], ALU.mult)
                nc.vector.tensor_tensor(ta[:, :], tp[0][:, :], tp[1][:, :], ALU.subtract)
                nc.vector.tensor_tensor(ta[:, :], ta[:, :], tp[2][:, :], ALU.add)
                nc.vector.tensor_tensor(ta[:, :], ta[:, :], tp[3][:, :], ALU.subtract)
                nc.vector.tensor_tensor(ta[:, :], ta[:, :], tp[4][:, :], ALU.add)
                nc.vector.tensor_tensor(Ar[:, :], ta[:, :], tp[5][:, :], ALU.subtract)
                # imag products
                nc.gpsimd.tensor_tensor(tp[0][:, :], X1[0][:, :], PA[1][:, :], ALU.mult)
                nc.gpsimd.tensor_tensor(tp[1][:, :], X1[1][:, :], PA[0][:, :], ALU.mult)
                nc.gpsimd.tensor_tensor(tp[2][:, :], X2[0][:, :], PB[1][:, :], ALU.mult)
                nc.vector.tensor_tensor(tp[3][:, :], X2[1][:, :], PB[0][:, :], ALU.mult)
                nc.vector.tensor_tensor(tp[4][:, :], X3[0][:, :], PC[1][:, :], ALU.mult)
                nc.vector.tensor_tensor(tp[5][:, :], X3[1][:, :], PC[0][:, :], ALU.mult)
                nc.vector.tensor_tensor(ta[:, :], tp[0][:, :], tp[1][:, :], ALU.add)
                nc.vector.tensor_tensor(ta[:, :], ta[:, :], tp[2][:, :], ALU.add)
                nc.vector.tensor_tensor(ta[:, :], ta[:, :], tp[3][:, :], ALU.add)
                nc.vector.tensor_tensor(ta[:, :], ta[:, :], tp[4][:, :], ALU.add)
                nc.vector.tensor_tensor(Ai[:, :], ta[:, :], tp[5][:, :], ALU.add)

                # --- normalize: out = sqrt(abs2) / max(sqrt(sum_c abs2), 1e-12) ---
                AB = pool.tile([P, TW], F32, name="AB", tag="AB")
                nc.vector.tensor_tensor(ta[:, :], Ar[:, :], Ar[:, :], ALU.mult)
                nc.vector.tensor_tensor(tb[:, :], Ai[:, :], Ai[:, :], ALU.mult)
                nc.vector.tensor_tensor(AB[:, :], ta[:, :], tb[:, :], ALU.add)
                r1 = scr.tile([P, 10 * T], F32, name="r1", tag="r1")
                nc.vector.tensor_tensor(r1[:, :], AB[:, 0:10 * T], AB[:, 10 * T:20 * T],
                                        ALU.add)
                r2 = scr.tile([P, 5 * T], F32, name="r2", tag="r2")
                nc.vector.tensor_tensor(r2[:, :], r1[:, 0:5 * T], r1[:, 5 * T:10 * T],
                                        ALU.add)
                r3 = scr.tile([P, 2 * T], F32, name="r3", tag="r3")
                nc.vector.tensor_tensor(r3[:, :], r2[:, 0:2 * T], r2[:, 2 * T:4 * T],
                                        ALU.add)
                tot = scr.tile([P, T], F32, name="tot", tag="tot")
                nc.vector.tensor_tensor(tot[:, :], r3[:, 0:T], r3[:, T:2 * T], ALU.add)
                nc.vector.tensor_tensor(tot[:, :], tot[:, :], r2[:, 4 * T:5 * T],
                                        ALU.add)
                sn = scr.tile([P, T], F32, name="sn", tag="sn")
                nc.scalar.activation(sn[:, :], tot[:, :], ACTF.Sqrt)
                nc.vector.tensor_scalar_max(sn[:, :], sn[:, :], 1e-12)
                rinv = scr.tile([P, T], F32, name="rinv", tag="rinv")
                nc.vector.reciprocal(rinv[:, :], sn[:, :])
                OUT = pool.tile([P, TW], F32, name="OUT", tag="OUT")
                nc.scalar.activation(OUT[:, :], AB[:, :], ACTF.Sqrt)
                rb = bass.AP(rinv[:, 0:1].tensor, rinv[:, 0:1].offset,
                             [rinv[:, 0:1].ap[0], [0, 20], [1, T]])
                # write scaled result transposed to (t, c) so one DMA suffices
                OUT2 = pool.tile([P, TW], F32, name="OUT2", tag="OUT2")
                o2 = OUT2[:, 0:1]
                out_tc = bass.AP(o2.tensor, o2.offset, [o2.ap[0], [1, 20],
                                                        [20, T]])
                nc.vector.tensor_tensor(out_tc, _blk(OUT, 0, 20), rb, ALU.mult)

                # --- DMA out: sbuf (p, c, t) -> dram [(p*32+t), c], per config c ---
                oa = out_ext[:, :]
                dst = bass.AP(oa.tensor, 0, [[20 * T, P], [1, 20 * T]])
                nc.sync.dma_start(out=dst, in_=OUT2[:, :])

    if split_waits:
        _split_excess_waits(nc)
    return nc


def _split_excess_waits(nc):
    """HW compute instructions hold at most 1 embedded sem-wait; Tile
    occasionally attaches 2. Hoist extras onto EventSemaphore insts (cap 2)."""
    nsplit = 0
    for f in nc.m.functions:
        for blk in f.blocks:
            new = []
            for inst in blk.instructions:
                si = inst.sync_info
                if (si is not None and len(si.on_wait) > 1
                        and type(inst).__name__ != "InstEventSemaphore"):
                    waits = list(si.on_wait)
                    keep, extra = waits[-1], waits[:-1]
                    while extra:
                        chunk, extra = extra[:2], extra[2:]
                        nsplit += 1
                        new.append(mybir.InstEventSemaphore(
                            name=f"{inst.name}-ws{nsplit}",
                            engine=inst.engine, ins=[], outs=[],
                            sync_info=mybir.SyncInfo(on_wait=chunk,
                                                     on_update=[])))
                    inst.sync_info = mybir.SyncInfo(
                        on_wait=[keep], on_update=list(si.on_update))
                new.append(inst)
            blk.instructions = new


_NC_CACHE = {}


def build_in_maps(x, params, output_phase, param_phi, param_theta,
                  input_k, input_b):
    x = np.ascontiguousarray(np.asarray(x, dtype=np.float32))
    d = _build_dvec(params, output_phase, param_phi, param_theta,
                    input_k, input_b)
    kfull = np.tile(d[IDX_K:IDX_K + 12], T).astype(np.float32)      # (t,j)
    bfull = np.tile(d[IDX_B:IDX_B + 12], T).astype(np.float32)
    tailrow = np.concatenate([d, kfull, bfull])                      # 96+768
    tail = np.tile(tailrow, (P, 1))
    s3 = _calc_start_cols(params, output_phase)
    vrow = np.zeros(36 * T, dtype=np.float32)
    for r in range(6):
        for c in range(3):
            vrow[(r * 3 + c) * T:(r * 3 + c + 1) * T] = np.real(s3[r, c])
            vrow[18 * T + (r * 3 + c) * T:18 * T + (r * 3 + c + 1) * T] =                 np.imag(s3[r, c])
    vtile = np.tile(vrow, (P, 1))
    in_maps = []
    for i in range(N_CORES):
        shard = x[i * SHARD:(i + 1) * SHARD].reshape(P, 12 * T)
        in_maps.append({
            "xd": np.ascontiguousarray(
                np.concatenate([shard, tail], axis=1)),
            "vinit": vtile,
        })
    return in_maps


def _make_callable(nc, n_cores=N_CORES):
    """Build a reusable jitted PJRT executable (avoids per-call NEFF upload)."""
    import jax
    from jax.sharding import Mesh, PartitionSpec
    from jax.experimental.shard_map import shard_map
    from concourse.bass2jax import (install_neuronx_cc_hook, _bass_exec_p,
                                    partition_id_tensor)
    install_neuronx_cc_hook()
    in_names, out_names, out_avals, zero_outs = [], [], [], []
    for alloc in nc.m.functions[0].allocations:
        if not isinstance(alloc, mybir.MemoryLocationSet):
            continue
        name = alloc.memorylocations[0].name
        if alloc.kind == "ExternalInput":
            if name != "partition_id":
                in_names.append(name)
        elif alloc.kind == "ExternalOutput":
            out_names.append(name)
            shape = tuple(alloc.tensor_shape)
            dtype = mybir.dt.np(alloc.dtype)
            out_avals.append(jax.core.ShapedArray(shape, dtype))
            zero_outs.append(np.zeros(shape, dtype))
    n_params = len(in_names)
    n_outs = len(out_avals)
    has_pid = nc.partition_id_tensor is not None
    all_in = in_names + out_names + (["partition_id"] if has_pid else [])

    def _body(*args):
        operands = list(args)
        if has_pid:
            operands.append(partition_id_tensor())
        outs = _bass_exec_p.bind(
            *operands, out_avals=tuple(out_avals), in_names=tuple(all_in),
            out_names=tuple(out_names), lowering_input_output_aliases=(),
            sim_require_finite=True, sim_require_nnan=True, nc=nc)
        return tuple(outs)

    devices = jax.devices()[:n_cores]
    mesh = Mesh(np.asarray(devices), ("core",))
    f = jax.jit(shard_map(_body, mesh=mesh,
                in_specs=(PartitionSpec("core"),) * (n_params + n_outs),
                out_specs=(PartitionSpec("core"),) * n_outs, check_rep=False),
                keep_unused=True)
    return f, in_names, zero_outs


def kernel(x, params, output_phase, param_phi, param_theta, input_k, input_b):
    if "f" not in _NC_CACHE:
        nc = build_kernel()
        _NC_CACHE["nc"] = nc
        _NC_CACHE["f"] = _make_callable(nc)
    f, in_names, zero_outs = _NC_CACHE["f"]
    in_maps = build_in_maps(x, params, output_phase, param_phi, param_theta,
                            input_k, input_b)
    gin = [np.concatenate([in_maps[c][n] for c in range(N_CORES)], axis=0)
           for n in in_names]
    gz = [np.zeros((N_CORES * z.shape[0], *z.shape[1:]), z.dtype)
          for z in zero_outs]
    out_arr = np.asarray(f(*(gin + gz))[0])
    return np.ascontiguousarray(out_arr.reshape(BATCH, 20)).astype(np.float32)

